# revision 12
# baseline (speedup 1.0000x reference)
"""2-layer GAT (DGL GATConv) on 8 TRN2 NeuronCores, batch-parallel.

Each core runs one batch element's full graph: N=5000 nodes, E=80000 edges,
128 -> 128 -> 64 features, edge softmax per destination node, final row
softmax.  Edges are sorted by dst on the host and padded into 128-edge
chunks grouped by 128-node destination blocks; segment reductions become
one-hot (fp8) x gathered-row (bf16) matmuls accumulated in PSUM.

Key restructure vs the naive formulation: layer 2 only consumes 66 linear
projections of the layer-1 output h (z2 = h@W2, el2 = h@W2@al2,
er2 = h@W2@ar2), and attention aggregation commutes with linear maps, so
W2 folds into the layer-1 gather table: L1 rows are
[u = z1@W2aug (66) | 1 | el1] = 68 bf16 cols -> one 256 B gather packet
(the dma_gather minimum), halving L1 gather HBM traffic vs gathering z1.
The L1 epilogue then emits layer-2's z2 rows directly (no L2 node phase,
no hT transposes); b1 propagates exactly through the attention average
(sum(alpha)=1) as the constant b1@W2aug added to the epilogue.
"""

import os
import sys
import numpy as np

sys.path.insert(0, "/opt/trn_rl_repo")

import ml_dtypes

import concourse.bass as bass
import concourse.mybir as mybir
from concourse import bacc, tile
from concourse.bass_utils import run_bass_kernel_spmd

BF16 = ml_dtypes.bfloat16
FP8 = ml_dtypes.float8_e4m3

N_NODES = 5000
N_EDGES = 80000
IN_DIM = 128
HID_DIM = 128
OUT_DIM = 64
BATCH = 8
NEG_SLOPE = 0.2
NB = (N_NODES + 127) // 128          # 40 node blocks
NPAD = NB * 128                      # 5120
SB_BLOCKS = 1                        # node blocks per superblock
NSTRIP = 8                           # resident one-hot load strips
RW = 128                             # bf16 row width for both gathers (256B)
F1 = OUT_DIM + 2                     # 66: aggregated cols in layer 1 (u)
F2 = OUT_DIM                         # 64: aggregated cols in layer 2

_CACHE = {}


# ----------------------------------------------------------------------------
# Host-side graph preprocessing (pure index manipulation)
# ----------------------------------------------------------------------------

def _prep_graph(src, dst):
    """Sort edges by dst, group into 128-node destination blocks, pad each
    block to a multiple of 128 edges, add one fake edge per padding node so
    every output row has a nonzero softmax denominator.

    Edge e lives at partition e%128, chunk e//128.
    """
    src = np.asarray(src).astype(np.int64).ravel()
    dst = np.asarray(dst).astype(np.int64).ravel()
    perm = np.argsort(dst, kind="stable")
    src_s, dst_s = src[perm], dst[perm]

    blocks_src = []
    blocks_oh = []
    blk_of_chunk = []
    for b in range(NB):
        lo, hi = b * 128, (b + 1) * 128
        sel = (dst_s >= lo) & (dst_s < hi)
        bs = src_s[sel]
        boh = dst_s[sel] - lo
        if b == NB - 1:
            # fake edges for padding nodes (N_NODES..NPAD-1): real one-hot
            # column so denom > 0, src index 0 (any valid node)
            npadnodes = NPAD - N_NODES
            bs = np.concatenate([bs, np.zeros(npadnodes, np.int64)])
            boh = np.concatenate(
                [boh, np.arange(N_NODES - lo, NPAD - lo, dtype=np.int64)]
            )
        nb_edges = len(bs)
        npad = (-nb_edges) % 128
        if npad:
            bs = np.concatenate([bs, np.zeros(npad, np.int64)])
            boh = np.concatenate([boh, -np.ones(npad, np.int64)])
        blocks_src.append(bs)
        blocks_oh.append(boh)
        blk_of_chunk.extend([b] * (len(bs) // 128))

    return {
        "src_pad": np.concatenate(blocks_src),
        "oh_col": np.concatenate(blocks_oh),
        "blk_of_chunk": np.asarray(blk_of_chunk, np.int64),
    }


def _host_arrays(src, dst):
    g = _prep_graph(src, dst)
    src_pad, oh_col = g["src_pad"], g["oh_col"]
    blk_of_chunk = g["blk_of_chunk"]
    E = len(src_pad)
    G = E // 128

    # dma_gather index layout: unwrapped i = s*16 + (p%16), replicated per core
    gidx = np.empty((128, E // 16), np.int16)
    for p16 in range(16):
        gidx[p16, :] = src_pad[p16::16]
    for c in range(1, 8):
        gidx[c * 16:(c + 1) * 16, :] = gidx[:16, :]

    # one-hot scatter matrices, per-partition-contiguous layout
    # S0[e, c*128 + d] = 1 if edge (c*128+e) has dst col d   (contract edges)
    # S0T[d, c*128 + e] = same nonzeros transposed            (contract dst)
    ohm = oh_col.reshape(G, 128).T  # [128 e, G]
    S0 = np.zeros((128, G * 128), FP8)
    cols = np.arange(G) * 128 + np.where(ohm >= 0, ohm, 0)
    rows = np.repeat(np.arange(128), G)
    vals = (ohm >= 0).astype(np.float32)
    S0[rows, cols.ravel()] = vals.ravel().astype(FP8)

    S0T = np.zeros((128, G * 128), FP8)
    flat_e = np.arange(G * 128)
    valid = oh_col >= 0
    S0T[oh_col[valid].astype(np.int64), flat_e[valid]] = 1.0

    # superblock chunk ranges (SB_BLOCKS node blocks each)
    sbs = []
    for b0 in range(0, NB, SB_BLOCKS):
        b1 = min(b0 + SB_BLOCKS, NB)
        chunks = np.nonzero((blk_of_chunk >= b0) & (blk_of_chunk < b1))[0]
        c0, c1 = int(chunks[0]), int(chunks[-1]) + 1
        blks = []
        for b in range(b0, b1):
            bc = np.nonzero(blk_of_chunk == b)[0]
            blks.append((b, int(bc[0]), int(bc[-1]) + 1))
        sbs.append((c0, c1, blks))
    maxch = max(c1 - c0 for c0, c1, _ in sbs)

    return {
        "G": G,
        "gidx": gidx,
        "S0": S0,
        "S0T": S0T,
        "sbs": sbs,
        "maxch": maxch,
        "blk_of_chunk": blk_of_chunk,
    }


# ----------------------------------------------------------------------------
# Device kernel builder
# ----------------------------------------------------------------------------

def _build_nc(G, sbs, maxch, blk_of_chunk):
    f32 = mybir.dt.float32
    bf16 = mybir.dt.bfloat16
    fp8 = mybir.dt.float8e4
    i16 = mybir.dt.int16
    AF = mybir.ActivationFunctionType
    ALU = mybir.AluOpType

    nc = bacc.Bacc("TRN2", target_bir_lowering=False, debug=False,
                   num_swdge_queues=4)

    # inputs (Waug1 = [W1@W2aug | W1@al1 | W1@ar1] precomputed on host)
    xT_d = nc.dram_tensor("xT", [128, NPAD], bf16, kind="ExternalInput")
    Waug1_d = nc.dram_tensor("Waug1", [128, F1 + 2], bf16,
                             kind="ExternalInput")
    b2_d = nc.dram_tensor("b2t", [128, OUT_DIM], f32, kind="ExternalInput")
    caug_d = nc.dram_tensor("caug", [128, OUT_DIM], f32, kind="ExternalInput")
    cattn_d = nc.dram_tensor("cattn", [128, 2], f32, kind="ExternalInput")
    S0_d = nc.dram_tensor("S0", [128, G * 128], fp8, kind="ExternalInput")
    S0T_d = nc.dram_tensor("S0T", [128, G * 128], fp8, kind="ExternalInput")
    gidx_d = nc.dram_tensor("gidx", [128, G * 8], i16, kind="ExternalInput")

    out_d = nc.dram_tensor("out", [N_NODES, OUT_DIM], f32, kind="ExternalOutput")

    # DRAM scratch (gathered-row tables)
    z1_d = nc.dram_tensor("z1rows", [NPAD, RW], bf16)
    z2_d = nc.dram_tensor("z2rows", [NPAD, RW], bf16)

    with tile.TileContext(nc) as tc:
        # --------------------------------------------------------------
        # persistent SBUF
        # --------------------------------------------------------------
        const = tc.alloc_tile_pool(name="const", bufs=1)
        xT = const.tile([128, NPAD], bf16, tag="xT")
        Waug1 = const.tile([128, F1 + 2], bf16, tag="Waug1")
        b2t = const.tile([128, OUT_DIM], f32, tag="b2t")
        caug = const.tile([128, OUT_DIM], f32, tag="caug")
        cattn = const.tile([128, 2], f32, tag="cattn")
        gidx = const.tile([128, G * 8], i16, tag="gidx")
        ercol1 = const.tile([128, NB], bf16, tag="ercol1")
        ercol2 = const.tile([128, NB], bf16, tag="ercol2")
        # one-hots resident in SBUF for both layers, loaded once in
        # NSTRIP strip tiles so early chunks don't wait on the full 10.6MB
        CPS = (G + NSTRIP - 1) // NSTRIP      # chunks per strip
        s0s = [const.tile([128, CPS * 128], fp8, tag=f"s0s{i}",
                          name=f"s0s{i}") for i in range(NSTRIP)]
        s0Ts = [const.tile([128, CPS * 128], fp8, tag=f"s0Ts{i}",
                           name=f"s0Ts{i}") for i in range(NSTRIP)]

        # strip-split the x load so node-phase matmuls start early
        for s in range(4):
            nc.sync.dma_start(
                out=xT[:, s * (NPAD // 4):(s + 1) * (NPAD // 4)],
                in_=xT_d[:, s * (NPAD // 4):(s + 1) * (NPAD // 4)],
            )
        nc.sync.dma_start(out=Waug1[:, :], in_=Waug1_d[:, :])
        nc.sync.dma_start(out=b2t[:, :], in_=b2_d[:, :])
        nc.sync.dma_start(out=caug[:, :], in_=caug_d[:, :])
        nc.sync.dma_start(out=cattn[:, :], in_=cattn_d[:, :])
        nc.sync.dma_start(out=gidx[:, :], in_=gidx_d[:, :])
        for i in range(NSTRIP):
            lo, hi = i * CPS * 128, min((i + 1) * CPS * 128, G * 128)
            nc.sync.dma_start(out=s0s[i][:, 0:hi - lo], in_=S0_d[:, lo:hi])
            nc.sync.dma_start(out=s0Ts[i][:, 0:hi - lo], in_=S0T_d[:, lo:hi])

        def s0_slice(g):
            i, off = g // CPS, (g % CPS) * 128
            return s0s[i][:, off:off + 128]

        def s0T_slice(g):
            i, off = g // CPS, (g % CPS) * 128
            return s0Ts[i][:, off:off + 128]

        # row-assembly pools persist across both layers so layer-1's
        # epilogue (which writes layer-2's rows) shares them
        npool = tc.alloc_tile_pool(name="nprow", bufs=3)
        npsum = tc.alloc_tile_pool(name="npps", bufs=2, space="PSUM")

        # ---- L1 node phase: u rows [u | 1 | el] + er column table ----
        for b in range(NB):
            pz = npsum.tile([128, F1 + 2], f32, tag="z")
            nc.tensor.matmul(
                pz[:, :], xT[:, b * 128:(b + 1) * 128], Waug1[:, :]
            )
            row = npool.tile([128, RW], bf16, tag="row")
            nc.scalar.copy(row[:, 0:F1], pz[:, 0:F1])
            nc.vector.memset(row[:, F1:F1 + 1], 1.0)
            nc.vector.tensor_copy(row[:, F1 + 1:F1 + 2], pz[:, F1:F1 + 1])
            nc.vector.tensor_copy(ercol1[:, b:b + 1], pz[:, F1 + 1:F1 + 2])
            nc.sync.dma_start(
                out=z1_d[b * 128:(b + 1) * 128, :], in_=row[:, :]
            )

        # --------------------------------------------------------------
        # edge phase (both layers; epilogues differ)
        # --------------------------------------------------------------
        def edge_phase(lidx, F, ercol, z_d):
            last = lidx == 2
            elc = F + 1    # el column in the gathered row
            with tc.tile_pool(name=f"zg{lidx}", bufs=2) as zgp, \
                 tc.tile_pool(name=f"ed{lidx}", bufs=3) as edp, \
                 tc.tile_pool(name=f"ep{lidx}", bufs=2) as epp, \
                 tc.tile_pool(name=f"erp{lidx}", bufs=2, space="PSUM") as erps, \
                 tc.tile_pool(name=f"agg{lidx}", bufs=4, space="PSUM") as apsum:
                for k, (c0, c1, blks) in enumerate(sbs):
                    nch = c1 - c0
                    zg = zgp.tile([128, maxch, RW], bf16, tag="zg")
                    # split desc-gen across the 4 SWDGE queues: each
                    # dma_gather only engages the Q7 core pair whose id
                    # matches queue_num, so 4 queues run concurrently
                    splits = [c0 + (nch * i) // 4 for i in range(4)] + [c1]
                    for q in range(4):
                        q0, q1 = splits[q], splits[q + 1]
                        if q1 == q0:
                            continue
                        ne = (q1 - q0) * 128
                        nc.gpsimd.dma_gather(
                            zg[:, q0 - c0:q1 - c0, :],
                            z_d[:, :],
                            gidx[:, q0 * 8:q1 * 8],
                            ne,
                            ne,
                            RW,
                            single_packet=False,
                            queue_num=q,
                        )
                    # er per edge: one-hot-transpose x er column
                    per = erps.tile([128, maxch], f32, tag="per")
                    for g in range(c0, c1):
                        bg = int(blk_of_chunk[g])
                        nc.tensor.matmul(
                            per[:, g - c0:g - c0 + 1],
                            s0T_slice(g),
                            ercol[:, bg:bg + 1],
                            start=True, stop=True,
                        )
                    # e = el + er ; lrelu ; exp  (all 2D APs: DVE chokes on
                    # degenerate [.., n, 1] shapes)
                    ee = edp.tile([128, maxch], f32, tag="ee")
                    nc.vector.tensor_tensor(
                        ee[:, 0:nch], per[:, 0:nch],
                        zg[:, 0:nch, elc], ALU.add
                    )
                    lr = edp.tile([128, maxch], f32, tag="lr")
                    nc.scalar.mul(lr[:, 0:nch], ee[:, 0:nch], NEG_SLOPE)
                    nc.vector.tensor_tensor(
                        lr[:, 0:nch], lr[:, 0:nch], ee[:, 0:nch], ALU.max
                    )
                    ex = edp.tile([128, maxch, 1], f32, tag="ex")
                    nc.scalar.activation(ex[:, 0:nch, :], lr[:, 0:nch], AF.Exp)
                    # scale gathered rows (incl ones column) by exp, one
                    # broadcast tensor_tensor per superblock
                    nc.vector.tensor_tensor(
                        zg[:, 0:nch, 0:F + 1],
                        zg[:, 0:nch, 0:F + 1],
                        ex[:, 0:nch, :].to_broadcast([128, nch, F + 1]),
                        ALU.mult,
                    )
                    # aggregate per node block
                    for (b, bc0, bc1) in blks:
                        pa = apsum.tile([128, F + 1], f32, tag="agg")
                        for g in range(bc0, bc1):
                            nc.tensor.matmul(
                                pa[:, :],
                                s0_slice(g),
                                zg[:, g - c0, 0:F + 1],
                                start=(g == bc0),
                                stop=(g == bc1 - 1),
                            )
                        # epilogue
                        rec = epp.tile([128, 1], f32, tag="rec")
                        nc.vector.reciprocal(rec[:, :], pa[:, F:F + 1])
                        if not last:
                            # z2aug = u_agg/denom + b1@W2aug; emit L2 rows
                            # [z2 | 1 | el2] and the er2 column directly
                            t = epp.tile([128, F1], f32, tag="t")
                            nc.scalar.mul(t[:, :], pa[:, 0:F1], rec[:, :])
                            row2 = npool.tile([128, RW], bf16, tag="row")
                            nc.vector.tensor_tensor(
                                row2[:, 0:F2], t[:, 0:F2], caug[:, :], ALU.add
                            )
                            nc.vector.memset(row2[:, F2:F2 + 1], 1.0)
                            nc.vector.tensor_scalar_add(
                                row2[:, F2 + 1:F2 + 2], t[:, F2:F2 + 1],
                                cattn[0:128, 0:1],
                            )
                            nc.vector.tensor_scalar_add(
                                ercol2[:, b:b + 1], t[:, F2 + 1:F2 + 2],
                                cattn[0:128, 1:2],
                            )
                            nc.sync.dma_start(
                                out=z2_d[b * 128:(b + 1) * 128, :],
                                in_=row2[:, :],
                            )
                        else:
                            os0 = epp.tile([128, OUT_DIM], f32, tag="os0")
                            nc.scalar.mul(os0[:, :], pa[:, 0:F2], rec[:, :])
                            osb = epp.tile([128, OUT_DIM], f32, tag="osb")
                            nc.vector.tensor_tensor(
                                osb[:, :], os0[:, :], b2t[:, :], ALU.add
                            )
                            mx = epp.tile([128, 1], f32, tag="mx")
                            nc.vector.tensor_reduce(
                                mx[:, :], osb[:, :],
                                axis=mybir.AxisListType.X,
                                op=ALU.max, negate=True,
                            )
                            eo = epp.tile([128, OUT_DIM], f32, tag="eo")
                            sden = epp.tile([128, 1], f32, tag="sden")
                            nc.scalar.activation(
                                eo[:, :], osb[:, :], AF.Exp,
                                bias=mx[:, :], accum_out=sden[:, :],
                            )
                            rec2 = epp.tile([128, 1], f32, tag="rec2")
                            nc.vector.reciprocal(rec2[:, :], sden[:, :])
                            ofin = epp.tile([128, OUT_DIM], f32, tag="ofin")
                            nc.scalar.mul(ofin[:, :], eo[:, :], rec2[:, :])
                            nrows = min(128, N_NODES - b * 128)
                            if nrows > 0:
                                nc.sync.dma_start(
                                    out=out_d[b * 128:b * 128 + nrows, :],
                                    in_=ofin[0:nrows, :],
                                )

        edge_phase(1, F1, ercol1, z1_d)
        edge_phase(2, F2, ercol2, z2_d)
        npool.release()
        npsum.release()
        const.release()

    nc.compile()
    return nc


# ----------------------------------------------------------------------------
# entry point
# ----------------------------------------------------------------------------

def _get_compiled(src, dst):
    key = (hash(np.asarray(src).tobytes()), hash(np.asarray(dst).tobytes()))
    if key not in _CACHE:
        host = _host_arrays(src, dst)
        nc = _build_nc(host["G"], host["sbs"], host["maxch"],
                       host["blk_of_chunk"])
        _CACHE[key] = (host, nc)
    return _CACHE[key]


def _make_in_maps(x, W1, al1, ar1, b1, W2, al2, ar2, b2, src, dst):
    host, nc = _get_compiled(src, dst)
    W1f = np.asarray(W1, np.float32)
    W2f = np.asarray(W2, np.float32)
    al1f = np.asarray(al1, np.float32).ravel()
    ar1f = np.asarray(ar1, np.float32).ravel()
    al2f = np.asarray(al2, np.float32).ravel()
    ar2f = np.asarray(ar2, np.float32).ravel()
    b1f = np.asarray(b1, np.float32).ravel()
    b2f = np.asarray(b2, np.float32).ravel()

    # W2 folded into the L1 table: u = z1 @ W2aug (66 cols), since layer 2
    # only consumes these projections of h and aggregation is linear
    W2aug = np.concatenate(
        [W2f, (W2f @ al2f)[:, None], (W2f @ ar2f)[:, None]], 1)  # [128, 66]
    Waug1 = np.concatenate(
        [W1f @ W2aug, (W1f @ al1f)[:, None], (W1f @ ar1f)[:, None]], 1
    ).astype(BF16)                                               # [128, 68]
    caugv = b1f @ W2aug                                          # [66]
    cattn = np.zeros((128, 2), np.float32)
    cattn[:, 0] = caugv[F2]       # c_el2
    cattn[:, 1] = caugv[F2 + 1]   # c_er2

    shared = {
        "Waug1": Waug1,
        "b2t": np.broadcast_to(b2f, (128, OUT_DIM)).copy(),
        "caug": np.broadcast_to(caugv[:F2], (128, OUT_DIM)).copy().astype(
            np.float32),
        "cattn": cattn,
        "S0": host["S0"],
        "S0T": host["S0T"],
        "gidx": host["gidx"],
    }
    xpad = np.zeros((BATCH, NPAD, IN_DIM), np.float32)
    xpad[:, :N_NODES, :] = np.asarray(x, np.float32)
    in_maps = [
        {**shared, "xT": np.ascontiguousarray(xpad[b].T).astype(BF16)}
        for b in range(BATCH)
    ]
    return nc, in_maps


def kernel(x, W1, al1, ar1, b1, W2, al2, ar2, b2, src, dst):
    nc, in_maps = _make_in_maps(x, W1, al1, ar1, b1, W2, al2, ar2, b2,
                                src, dst)
    res = run_bass_kernel_spmd(nc, in_maps, list(range(BATCH)))
    out = np.stack([res.results[b]["out"] for b in range(BATCH)])
    return out.reshape(BATCH * N_NODES, OUT_DIM).astype(np.float32)


def run_timed(x, W1, al1, ar1, b1, W2, al2, ar2, b2, src, dst, **kw):
    """Run with NTFF profiling; returns exec_time_ns (or None)."""
    nc, in_maps = _make_in_maps(x, W1, al1, ar1, b1, W2, al2, ar2, b2,
                                src, dst)
    res = run_bass_kernel_spmd(nc, in_maps, list(range(BATCH)), trace=True)
    return res.exec_time_ns


# revision 19
# speedup vs baseline: 1.4687x; 1.4687x over previous
"""2-layer GAT (DGL GATConv) on 8 TRN2 NeuronCores, batch-parallel.

Each core runs one batch element's full graph: N=5000 nodes, E=80000 edges,
128 -> 128 -> 64 features, edge softmax per destination node, final row
softmax.  Edges are sorted by dst on the host and padded into 128-edge
chunks grouped by 128-node destination blocks; segment reductions become
one-hot (fp8) x gathered-row (bf16) matmuls accumulated in PSUM.

Key restructure vs the naive formulation: layer 2 only consumes 66 linear
projections of the layer-1 output h (z2 = h@W2, el2 = h@W2@al2,
er2 = h@W2@ar2), and attention aggregation commutes with linear maps, so
W2 folds into the layer-1 gather table: L1 rows are
[u = z1@W2aug (66) | 1 | el1] = 68 bf16 cols -> one 256 B gather packet
(the dma_gather minimum), halving L1 gather HBM traffic vs gathering z1.
The L1 epilogue then emits layer-2's z2 rows directly (no L2 node phase,
no hT transposes); b1 propagates exactly through the attention average
(sum(alpha)=1) as the constant b1@W2aug added to the epilogue.
"""

import os
import sys
import numpy as np

sys.path.insert(0, "/opt/trn_rl_repo")

import ml_dtypes

import concourse.bass as bass
import concourse.mybir as mybir
from concourse import bacc, tile
from concourse.bass_utils import run_bass_kernel_spmd

BF16 = ml_dtypes.bfloat16
FP8 = ml_dtypes.float8_e4m3

N_NODES = 5000
N_EDGES = 80000
IN_DIM = 128
HID_DIM = 128
OUT_DIM = 64
BATCH = 8
NEG_SLOPE = 0.2
NB = (N_NODES + 127) // 128          # 40 node blocks
NPAD = NB * 128                      # 5120
SB_BLOCKS = 2                        # node blocks per superblock
NSTRIP = 8                           # resident one-hot load strips
RW = 128                             # bf16 row width for both gathers (256B)
F1 = OUT_DIM + 2                     # 66: aggregated cols in layer 1 (u)
F2 = OUT_DIM                         # 64: aggregated cols in layer 2

_CACHE = {}


# ----------------------------------------------------------------------------
# Host-side graph preprocessing (pure index manipulation)
# ----------------------------------------------------------------------------

def _prep_graph(src, dst):
    """Sort edges by dst, group into 128-node destination blocks, pad each
    block to a multiple of 128 edges, add one fake edge per padding node so
    every output row has a nonzero softmax denominator.

    Edge e lives at partition e%128, chunk e//128.
    """
    src = np.asarray(src).astype(np.int64).ravel()
    dst = np.asarray(dst).astype(np.int64).ravel()
    perm = np.argsort(dst, kind="stable")
    src_s, dst_s = src[perm], dst[perm]

    blocks_src = []
    blocks_oh = []
    blk_of_chunk = []
    for b in range(NB):
        lo, hi = b * 128, (b + 1) * 128
        sel = (dst_s >= lo) & (dst_s < hi)
        bs = src_s[sel]
        boh = dst_s[sel] - lo
        if b == NB - 1:
            # fake edges for padding nodes (N_NODES..NPAD-1): real one-hot
            # column so denom > 0, src index 0 (any valid node)
            npadnodes = NPAD - N_NODES
            bs = np.concatenate([bs, np.zeros(npadnodes, np.int64)])
            boh = np.concatenate(
                [boh, np.arange(N_NODES - lo, NPAD - lo, dtype=np.int64)]
            )
        nb_edges = len(bs)
        npad = (-nb_edges) % 128
        if npad:
            bs = np.concatenate([bs, np.zeros(npad, np.int64)])
            boh = np.concatenate([boh, -np.ones(npad, np.int64)])
        blocks_src.append(bs)
        blocks_oh.append(boh)
        blk_of_chunk.extend([b] * (len(bs) // 128))

    return {
        "src_pad": np.concatenate(blocks_src),
        "oh_col": np.concatenate(blocks_oh),
        "blk_of_chunk": np.asarray(blk_of_chunk, np.int64),
    }


def _host_arrays(src, dst):
    g = _prep_graph(src, dst)
    src_pad, oh_col = g["src_pad"], g["oh_col"]
    blk_of_chunk = g["blk_of_chunk"]
    E = len(src_pad)
    G = E // 128

    # dma_gather index layout: unwrapped i = s*16 + (p%16), replicated per core
    gidx = np.empty((128, E // 16), np.int16)
    for p16 in range(16):
        gidx[p16, :] = src_pad[p16::16]
    for c in range(1, 8):
        gidx[c * 16:(c + 1) * 16, :] = gidx[:16, :]

    # one-hot scatter matrices, per-partition-contiguous layout
    # S0[e, c*128 + d] = 1 if edge (c*128+e) has dst col d   (contract edges)
    # S0T[d, c*128 + e] = same nonzeros transposed            (contract dst)
    ohm = oh_col.reshape(G, 128).T  # [128 e, G]
    S0 = np.zeros((128, G * 128), FP8)
    cols = np.arange(G) * 128 + np.where(ohm >= 0, ohm, 0)
    rows = np.repeat(np.arange(128), G)
    vals = (ohm >= 0).astype(np.float32)
    S0[rows, cols.ravel()] = vals.ravel().astype(FP8)

    S0T = np.zeros((128, G * 128), FP8)
    flat_e = np.arange(G * 128)
    valid = oh_col >= 0
    S0T[oh_col[valid].astype(np.int64), flat_e[valid]] = 1.0

    # superblock chunk ranges (SB_BLOCKS node blocks each)
    sbs = []
    for b0 in range(0, NB, SB_BLOCKS):
        b1 = min(b0 + SB_BLOCKS, NB)
        chunks = np.nonzero((blk_of_chunk >= b0) & (blk_of_chunk < b1))[0]
        c0, c1 = int(chunks[0]), int(chunks[-1]) + 1
        blks = []
        for b in range(b0, b1):
            bc = np.nonzero(blk_of_chunk == b)[0]
            blks.append((b, int(bc[0]), int(bc[-1]) + 1))
        sbs.append((c0, c1, blks))
    maxch = max(c1 - c0 for c0, c1, _ in sbs)

    return {
        "G": G,
        "gidx": gidx,
        "S0": S0,
        "S0T": S0T,
        "sbs": sbs,
        "maxch": maxch,
        "blk_of_chunk": blk_of_chunk,
    }


# ----------------------------------------------------------------------------
# Device kernel builder
# ----------------------------------------------------------------------------

def _build_nc(G, sbs, maxch, blk_of_chunk):
    f32 = mybir.dt.float32
    bf16 = mybir.dt.bfloat16
    fp8 = mybir.dt.float8e4
    i16 = mybir.dt.int16
    AF = mybir.ActivationFunctionType
    ALU = mybir.AluOpType

    nc = bacc.Bacc("TRN2", target_bir_lowering=False, debug=False,
                   num_swdge_queues=4, dynamic_dma_scratch_size=49152)

    # inputs (Waug1 = [W1@W2aug | W1@al1 | W1@ar1] precomputed on host)
    xT_d = nc.dram_tensor("xT", [128, NPAD], bf16, kind="ExternalInput")
    Waug1_d = nc.dram_tensor("Waug1", [128, F1 + 2], bf16,
                             kind="ExternalInput")
    b2_d = nc.dram_tensor("b2t", [128, OUT_DIM], f32, kind="ExternalInput")
    caug_d = nc.dram_tensor("caug", [128, OUT_DIM], f32, kind="ExternalInput")
    cattn_d = nc.dram_tensor("cattn", [128, 2], f32, kind="ExternalInput")
    S0_d = nc.dram_tensor("S0", [128, G * 128], fp8, kind="ExternalInput")
    S0T_d = nc.dram_tensor("S0T", [128, G * 128], fp8, kind="ExternalInput")
    gidx_d = nc.dram_tensor("gidx", [128, G * 8], i16, kind="ExternalInput")

    out_d = nc.dram_tensor("out", [N_NODES, OUT_DIM], f32, kind="ExternalOutput")

    # DRAM scratch (gathered-row tables)
    z1_d = nc.dram_tensor("z1rows", [NPAD, RW], bf16)
    z2_d = nc.dram_tensor("z2rows", [NPAD, RW], bf16)

    with tile.TileContext(nc) as tc:
        # --------------------------------------------------------------
        # persistent SBUF
        # --------------------------------------------------------------
        const = tc.alloc_tile_pool(name="const", bufs=1)
        xT = const.tile([128, NPAD], bf16, tag="xT")
        Waug1 = const.tile([128, F1 + 2], bf16, tag="Waug1")
        b2t = const.tile([128, OUT_DIM], f32, tag="b2t")
        caug = const.tile([128, OUT_DIM], f32, tag="caug")
        cattn = const.tile([128, 2], f32, tag="cattn")
        gidx = const.tile([128, G * 8], i16, tag="gidx")
        ercol1 = const.tile([128, NB], bf16, tag="ercol1")
        ercol2 = const.tile([128, NB], bf16, tag="ercol2")
        # er-side one-hot resident in SBUF for both layers, loaded once in
        # NSTRIP strip tiles so early chunks don't wait on the full 10.6MB
        CPS = (G + NSTRIP - 1) // NSTRIP      # chunks per strip
        s0Ts = [const.tile([128, CPS * 128], fp8, tag=f"s0Ts{i}",
                           name=f"s0Ts{i}") for i in range(NSTRIP)]

        # strip-split the x load so node-phase matmuls start early
        for s in range(4):
            nc.sync.dma_start(
                out=xT[:, s * (NPAD // 4):(s + 1) * (NPAD // 4)],
                in_=xT_d[:, s * (NPAD // 4):(s + 1) * (NPAD // 4)],
            )
        nc.sync.dma_start(out=Waug1[:, :], in_=Waug1_d[:, :])
        nc.sync.dma_start(out=b2t[:, :], in_=b2_d[:, :])
        nc.sync.dma_start(out=caug[:, :], in_=caug_d[:, :])
        nc.sync.dma_start(out=cattn[:, :], in_=cattn_d[:, :])
        nc.sync.dma_start(out=gidx[:, :], in_=gidx_d[:, :])
        for i in range(NSTRIP):
            lo, hi = i * CPS * 128, min((i + 1) * CPS * 128, G * 128)
            nc.sync.dma_start(out=s0Ts[i][:, 0:hi - lo], in_=S0T_d[:, lo:hi])

        def s0T_slice(g):
            i, off = g // CPS, (g % CPS) * 128
            return s0Ts[i][:, off:off + 128]

        # row-assembly pools persist across both layers so layer-1's
        # epilogue (which writes layer-2's rows) shares them
        npool = tc.alloc_tile_pool(name="nprow", bufs=3)
        npsum = tc.alloc_tile_pool(name="npps", bufs=2, space="PSUM")

        # ---- L1 node phase: u rows [u | 1 | el] + er column table ----
        for b in range(NB):
            pz = npsum.tile([128, F1 + 2], f32, tag="z")
            nc.tensor.matmul(
                pz[:, :], xT[:, b * 128:(b + 1) * 128], Waug1[:, :]
            )
            row = npool.tile([128, RW], bf16, tag="row")
            nc.scalar.copy(row[:, 0:F1], pz[:, 0:F1])
            nc.vector.memset(row[:, F1:F1 + 1], 1.0)
            nc.vector.tensor_copy(row[:, F1 + 1:F1 + 2], pz[:, F1:F1 + 1])
            nc.vector.tensor_copy(ercol1[:, b:b + 1], pz[:, F1 + 1:F1 + 2])
            nc.sync.dma_start(
                out=z1_d[b * 128:(b + 1) * 128, :], in_=row[:, :]
            )

        # --------------------------------------------------------------
        # edge phase (both layers; epilogues differ)
        # --------------------------------------------------------------
        def edge_phase(lidx, F, ercol, z_d):
            last = lidx == 2
            elc = F + 1    # el column in the gathered row
            with tc.tile_pool(name=f"zg{lidx}", bufs=4) as zgp, \
                 tc.tile_pool(name=f"s0{lidx}", bufs=3) as s0p, \
                 tc.tile_pool(name=f"ed{lidx}", bufs=3) as edp, \
                 tc.tile_pool(name=f"ep{lidx}", bufs=2) as epp, \
                 tc.tile_pool(name=f"erp{lidx}", bufs=2, space="PSUM") as erps, \
                 tc.tile_pool(name=f"agg{lidx}", bufs=4, space="PSUM") as apsum:
                for k, (c0, c1, blks) in enumerate(sbs):
                    nch = c1 - c0
                    zg = zgp.tile([128, maxch, RW], bf16, tag="zg")
                    # split desc-gen across the 4 SWDGE queues: each
                    # dma_gather only engages the Q7 core pair whose id
                    # matches queue_num, so 4 queues run concurrently
                    splits = [c0 + (nch * i) // 4 for i in range(4)] + [c1]
                    for q in range(4):
                        q0, q1 = splits[q], splits[q + 1]
                        if q1 == q0:
                            continue
                        ne = (q1 - q0) * 128
                        nc.gpsimd.dma_gather(
                            zg[:, q0 - c0:q1 - c0, :],
                            z_d[:, :],
                            gidx[:, q0 * 8:q1 * 8],
                            ne,
                            ne,
                            RW,
                            single_packet=False,
                            queue_num=q,
                        )
                    s0t = s0p.tile([128, maxch * 128], fp8, tag="s0")
                    nc.sync.dma_start(
                        out=s0t[:, 0:nch * 128],
                        in_=S0_d[:, c0 * 128:c1 * 128],
                    )
                    # er per edge: one-hot-transpose x er column
                    per = erps.tile([128, maxch], f32, tag="per")
                    for g in range(c0, c1):
                        bg = int(blk_of_chunk[g])
                        nc.tensor.matmul(
                            per[:, g - c0:g - c0 + 1],
                            s0T_slice(g),
                            ercol[:, bg:bg + 1],
                            start=True, stop=True,
                        )
                    # e = el + er ; lrelu ; exp  (all 2D APs: DVE chokes on
                    # degenerate [.., n, 1] shapes)
                    ee = edp.tile([128, maxch], f32, tag="ee")
                    nc.vector.tensor_tensor(
                        ee[:, 0:nch], per[:, 0:nch],
                        zg[:, 0:nch, elc], ALU.add
                    )
                    lr = edp.tile([128, maxch], f32, tag="lr")
                    nc.scalar.mul(lr[:, 0:nch], ee[:, 0:nch], NEG_SLOPE)
                    nc.vector.tensor_tensor(
                        lr[:, 0:nch], lr[:, 0:nch], ee[:, 0:nch], ALU.max
                    )
                    ex = edp.tile([128, maxch, 1], f32, tag="ex")
                    nc.scalar.activation(ex[:, 0:nch, :], lr[:, 0:nch], AF.Exp)
                    # scale gathered rows (incl ones column) by exp, one
                    # broadcast tensor_tensor per superblock
                    nc.vector.tensor_tensor(
                        zg[:, 0:nch, 0:F + 1],
                        zg[:, 0:nch, 0:F + 1],
                        ex[:, 0:nch, :].to_broadcast([128, nch, F + 1]),
                        ALU.mult,
                    )
                    # aggregate per node block
                    for (b, bc0, bc1) in blks:
                        pa = apsum.tile([128, F + 1], f32, tag="agg")
                        for g in range(bc0, bc1):
                            nc.tensor.matmul(
                                pa[:, :],
                                s0t[:, (g - c0) * 128:(g - c0 + 1) * 128],
                                zg[:, g - c0, 0:F + 1],
                                start=(g == bc0),
                                stop=(g == bc1 - 1),
                            )
                        # epilogue
                        rec = epp.tile([128, 1], f32, tag="rec")
                        nc.vector.reciprocal(rec[:, :], pa[:, F:F + 1])
                        if not last:
                            # z2aug = u_agg/denom + b1@W2aug; emit L2 rows
                            # [z2 | 1 | el2] and the er2 column directly
                            t = epp.tile([128, F1], f32, tag="t")
                            nc.scalar.mul(t[:, :], pa[:, 0:F1], rec[:, :])
                            row2 = npool.tile([128, RW], bf16, tag="row")
                            nc.vector.tensor_tensor(
                                row2[:, 0:F2], t[:, 0:F2], caug[:, :], ALU.add
                            )
                            nc.vector.memset(row2[:, F2:F2 + 1], 1.0)
                            nc.vector.tensor_scalar_add(
                                row2[:, F2 + 1:F2 + 2], t[:, F2:F2 + 1],
                                cattn[0:128, 0:1],
                            )
                            nc.vector.tensor_scalar_add(
                                ercol2[:, b:b + 1], t[:, F2 + 1:F2 + 2],
                                cattn[0:128, 1:2],
                            )
                            nc.sync.dma_start(
                                out=z2_d[b * 128:(b + 1) * 128, :],
                                in_=row2[:, :],
                            )
                        else:
                            os0 = epp.tile([128, OUT_DIM], f32, tag="os0")
                            nc.scalar.mul(os0[:, :], pa[:, 0:F2], rec[:, :])
                            osb = epp.tile([128, OUT_DIM], f32, tag="osb")
                            nc.vector.tensor_tensor(
                                osb[:, :], os0[:, :], b2t[:, :], ALU.add
                            )
                            mx = epp.tile([128, 1], f32, tag="mx")
                            nc.vector.tensor_reduce(
                                mx[:, :], osb[:, :],
                                axis=mybir.AxisListType.X,
                                op=ALU.max, negate=True,
                            )
                            eo = epp.tile([128, OUT_DIM], f32, tag="eo")
                            sden = epp.tile([128, 1], f32, tag="sden")
                            nc.scalar.activation(
                                eo[:, :], osb[:, :], AF.Exp,
                                bias=mx[:, :], accum_out=sden[:, :],
                            )
                            rec2 = epp.tile([128, 1], f32, tag="rec2")
                            nc.vector.reciprocal(rec2[:, :], sden[:, :])
                            ofin = epp.tile([128, OUT_DIM], f32, tag="ofin")
                            nc.scalar.mul(ofin[:, :], eo[:, :], rec2[:, :])
                            nrows = min(128, N_NODES - b * 128)
                            if nrows > 0:
                                nc.sync.dma_start(
                                    out=out_d[b * 128:b * 128 + nrows, :],
                                    in_=ofin[0:nrows, :],
                                )

        edge_phase(1, F1, ercol1, z1_d)
        edge_phase(2, F2, ercol2, z2_d)
        npool.release()
        npsum.release()
        const.release()

    nc.compile()
    return nc


# ----------------------------------------------------------------------------
# entry point
# ----------------------------------------------------------------------------

def _get_compiled(src, dst):
    key = (hash(np.asarray(src).tobytes()), hash(np.asarray(dst).tobytes()))
    if key not in _CACHE:
        host = _host_arrays(src, dst)
        nc = _build_nc(host["G"], host["sbs"], host["maxch"],
                       host["blk_of_chunk"])
        _CACHE[key] = (host, nc)
    return _CACHE[key]


def _make_in_maps(x, W1, al1, ar1, b1, W2, al2, ar2, b2, src, dst):
    host, nc = _get_compiled(src, dst)
    W1f = np.asarray(W1, np.float32)
    W2f = np.asarray(W2, np.float32)
    al1f = np.asarray(al1, np.float32).ravel()
    ar1f = np.asarray(ar1, np.float32).ravel()
    al2f = np.asarray(al2, np.float32).ravel()
    ar2f = np.asarray(ar2, np.float32).ravel()
    b1f = np.asarray(b1, np.float32).ravel()
    b2f = np.asarray(b2, np.float32).ravel()

    # W2 folded into the L1 table: u = z1 @ W2aug (66 cols), since layer 2
    # only consumes these projections of h and aggregation is linear
    W2aug = np.concatenate(
        [W2f, (W2f @ al2f)[:, None], (W2f @ ar2f)[:, None]], 1)  # [128, 66]
    Waug1 = np.concatenate(
        [W1f @ W2aug, (W1f @ al1f)[:, None], (W1f @ ar1f)[:, None]], 1
    ).astype(BF16)                                               # [128, 68]
    caugv = b1f @ W2aug                                          # [66]
    cattn = np.zeros((128, 2), np.float32)
    cattn[:, 0] = caugv[F2]       # c_el2
    cattn[:, 1] = caugv[F2 + 1]   # c_er2

    shared = {
        "Waug1": Waug1,
        "b2t": np.broadcast_to(b2f, (128, OUT_DIM)).copy(),
        "caug": np.broadcast_to(caugv[:F2], (128, OUT_DIM)).copy().astype(
            np.float32),
        "cattn": cattn,
        "S0": host["S0"],
        "S0T": host["S0T"],
        "gidx": host["gidx"],
    }
    xpad = np.zeros((BATCH, NPAD, IN_DIM), np.float32)
    xpad[:, :N_NODES, :] = np.asarray(x, np.float32)
    in_maps = [
        {**shared, "xT": np.ascontiguousarray(xpad[b].T).astype(BF16)}
        for b in range(BATCH)
    ]
    return nc, in_maps


def kernel(x, W1, al1, ar1, b1, W2, al2, ar2, b2, src, dst):
    nc, in_maps = _make_in_maps(x, W1, al1, ar1, b1, W2, al2, ar2, b2,
                                src, dst)
    res = run_bass_kernel_spmd(nc, in_maps, list(range(BATCH)))
    out = np.stack([res.results[b]["out"] for b in range(BATCH)])
    return out.reshape(BATCH * N_NODES, OUT_DIM).astype(np.float32)


def run_timed(x, W1, al1, ar1, b1, W2, al2, ar2, b2, src, dst, **kw):
    """Run with NTFF profiling; returns exec_time_ns (or None)."""
    nc, in_maps = _make_in_maps(x, W1, al1, ar1, b1, W2, al2, ar2, b2,
                                src, dst)
    res = run_bass_kernel_spmd(nc, in_maps, list(range(BATCH)), trace=True)
    return res.exec_time_ns
